# revision 13
# baseline (speedup 1.0000x reference)
"""Tensor-parallel causal multi-head attention (RoPE) on 8 TRN2 NeuronCores.

Sharding: heads are split across the 8 cores (16 heads -> 2 heads/core).
wq/wk/wv are split column-wise (by output head), wo row-wise; hidden_states
is replicated.  Each core computes its 2 heads end-to-end (QKV projection,
RoPE, causal attention, output projection) and returns its additive partial
of the full output; the host sums the 8 partials.

Device-side layout choices (all matmuls contract over the partition dim):
  - X^T [HID, B*S] is produced on the host so projections need no on-device
    transposes.  Q and K are computed directly in transposed layout
    Q^T/K^T [d, s] (lhsT = W^T chunk, rhs = X^T chunk), V in normal layout
    [s, d] (lhsT = X^T chunk, rhs = W^T).
  - Scores are computed transposed: S^T[k, q] = (K^T chunk).T @ Q^T, so the
    exp'd probabilities P^T [k, q] feed the O^T = V.T @ P^T matmul directly
    with q as the 512-wide moving dim (full fp32r rate), no transposes.
  - softmax denominators l[q] = sum_k P^T[k, q] come from a ones-column
    matmul accumulated alongside O^T; 1/l is broadcast across partitions
    with a rank-1 fp32 matmul (ones_row.T @ recip_l).
  - No max-subtraction: scores are O(1) for this problem so exp is safe.
  - RoPE's rotate_half is a partition swap done with two SBUF->SBUF DMAs;
    the sign flip is folded into the host-prepared sin^T (lower half
    negated), and the 1/sqrt(D) score scale is folded into wq.
"""

import math

import numpy as np

import concourse.bass as bass
import concourse.tile as tile
from concourse import bacc, mybir
from concourse.bass_utils import run_bass_kernel_spmd

B, S, HID = 2, 2048, 2048
H, D = 16, 128
NCORES = 8
HPC = H // NCORES  # heads per core
DH = HPC * D  # per-core projection width (256)
NHC = HID // 128  # hid chunks (16)
TS = 512  # s-tile for projections
TQ = 512  # q-tile for attention
NKB = S // 128  # k blocks per sequence (16)
F32 = mybir.dt.float32
F32R = mybir.dt.float32r

LAST_EXEC_TIME_NS = None
_CACHE = {}


def _build_device_program():
    nc = bacc.Bacc(
        "TRN2",
        target_bir_lowering=False,
        debug=False,
        enable_asserts=False,
        num_devices=NCORES,
    )
    xT = nc.dram_tensor("xT", [HID, B * S], F32R, kind="ExternalInput").ap()
    wqT = nc.dram_tensor("wqT", [HID, DH], F32R, kind="ExternalInput").ap()
    wkT = nc.dram_tensor("wkT", [HID, DH], F32R, kind="ExternalInput").ap()
    wvT = nc.dram_tensor("wvT", [HID, DH], F32R, kind="ExternalInput").ap()
    woT = nc.dram_tensor("woT", [DH, HID], F32R, kind="ExternalInput").ap()
    cosT = nc.dram_tensor("cosT", [D, B * S], F32, kind="ExternalInput").ap()
    sinT = nc.dram_tensor("sinT", [D, B * S], F32, kind="ExternalInput").ap()
    out = nc.dram_tensor("out", [B * S, HID], F32, kind="ExternalOutput").ap()

    with tile.TileContext(nc) as tc:
        _emit_kernel(tc, xT, wqT, wkT, wvT, woT, cosT, sinT, out)

    nc.compile()
    return nc


def _emit_kernel(tc, xT, wqT, wkT, wvT, woT, cosT, sinT, out, dbg=None):
    from contextlib import ExitStack

    nc = tc.nc
    with ExitStack() as ctx:
        xTr = xT.rearrange("(hc p) s -> p hc s", p=128)  # [128, 16, B*S]
        wqTr = wqT.rearrange("(hc p) d -> p hc d", p=128)  # [128, 16, DH]
        wkTr = wkT.rearrange("(hc p) d -> p hc d", p=128)
        wvTr = wvT.rearrange("(hc p) d -> p hc d", p=128)
        woTr = woT.rearrange("(wc p) e -> p wc e", p=128)  # [128, HPC, HID]

        const = ctx.enter_context(tc.tile_pool(name="const", bufs=1))
        batchp = ctx.enter_context(tc.tile_pool(name="batchp", bufs=1))
        xtp = ctx.enter_context(tc.tile_pool(name="xtp", bufs=2))
        csp = ctx.enter_context(tc.tile_pool(name="csp", bufs=2))
        tmpp = ctx.enter_context(tc.tile_pool(name="tmpp", bufs=8))
        ptp = ctx.enter_context(tc.tile_pool(name="ptp", bufs=4))
        recp = ctx.enter_context(tc.tile_pool(name="recp", bufs=2))
        psump = ctx.enter_context(tc.tile_pool(name="psump", bufs=8, space="PSUM"))

        # ---- resident constants ----
        # split weight loads per hid-chunk group so the first matmuls only
        # wait for the chunks they read; wo is loaded later (phase C)
        wq_sb = const.tile([128, NHC, DH], F32R)
        wk_sb = const.tile([128, NHC, DH], F32R)
        wv_sb = const.tile([128, NHC, DH], F32R)
        for j in range(8):
            c0, c1 = j * 2, j * 2 + 2
            nc.scalar.dma_start(out=wq_sb[:, c0:c1, :], in_=wqTr[:, c0:c1, :])
            nc.scalar.dma_start(out=wk_sb[:, c0:c1, :], in_=wkTr[:, c0:c1, :])
            nc.scalar.dma_start(out=wv_sb[:, c0:c1, :], in_=wvTr[:, c0:c1, :])
        wo_sb = const.tile([128, HPC, HID], F32R)
        ones_f = const.tile([128, 1], F32)
        nc.vector.memset(ones_f[:], 1.0)
        ones_col = const.tile([128, 1], F32R)
        nc.scalar.copy(ones_col[:], ones_f[:])

        for b in range(B):
            bs = b * S
            # per-batch on-chip tensors (slots shared across batches via tags)
            qt_sb = batchp.tile([128, HPC, S], F32R, tag="qt")  # Q^T (scaled, roped)
            kt_sb = batchp.tile([128, HPC, S], F32R, tag="kt")  # K^T (roped)
            v_sb = batchp.tile([128, NKB * DH], F32R, tag="v")  # V row-blocks
            at_sb = batchp.tile([128, HPC, S], F32R, tag="at")  # attn out (A^T)

            # ---- phase A: QKV projections + RoPE ----
            for st in range(S // TS):
                s0 = st * TS
                psq = [
                    psump.tile([128, TS], F32, tag="big", name=f"psq{h}")
                    for h in range(HPC)
                ]
                psk = [
                    psump.tile([128, TS], F32, tag="big", name=f"psk{h}")
                    for h in range(HPC)
                ]
                psv = [
                    psump.tile([128, TS], F32, tag="big", name=f"psv{sp}")
                    for sp in range(TS // 256)
                ]
                for half in range(2):
                    xt = xtp.tile([128, 8, TS], F32R)
                    for xj in range(2):
                        nc.sync.dma_start(
                            out=xt[:, xj * 4 : xj * 4 + 4, :],
                            in_=xTr[
                                :,
                                half * 8 + xj * 4 : half * 8 + xj * 4 + 4,
                                bs + s0 : bs + s0 + TS,
                            ],
                        )
                    for i in range(8):
                        hc = half * 8 + i
                        first = hc == 0
                        last = hc == NHC - 1
                        for h in range(HPC):
                            nc.tensor.matmul(
                                psq[h][:],
                                lhsT=(wq_sb[:, hc, h * D : (h + 1) * D]),
                                rhs=(xt[:, i, :]),
                                start=first,
                                stop=last,
                            )
                            nc.tensor.matmul(
                                psk[h][:],
                                lhsT=(wk_sb[:, hc, h * D : (h + 1) * D]),
                                rhs=(xt[:, i, :]),
                                start=first,
                                stop=last,
                            )
                        for sp in range(TS // 256):
                            for sblk in range(2):
                                # one accumulation group per PSUM bank:
                                # start=True clears the whole bank, so only
                                # the first matmul touching the tile starts
                                nc.tensor.matmul(
                                    psv[sp][:, sblk * DH : (sblk + 1) * DH],
                                    lhsT=(
                                        xt[:, i, (sp * 2 + sblk) * 128 : (sp * 2 + sblk + 1) * 128]
                                    ),
                                    rhs=(wv_sb[:, hc, :]),
                                    start=first and sblk == 0,
                                    stop=last and sblk == 1,
                                    skip_group_check=True,
                                )
                # V: evacuate PSUM -> v_sb
                for sp in range(TS // 256):
                    blk0 = s0 // 128 + sp * 2
                    nc.scalar.copy(
                        v_sb[:, blk0 * DH : (blk0 + 2) * DH], psv[sp][:]
                    )
                # RoPE for Q and K
                cs = csp.tile([128, TS], F32, tag="cs")
                nc.sync.dma_start(out=cs[:], in_=cosT[:, bs + s0 : bs + s0 + TS])
                sn = csp.tile([128, TS], F32, tag="cs")
                nc.sync.dma_start(out=sn[:], in_=sinT[:, bs + s0 : bs + s0 + TS])
                for ps_list, dst in ((psq, qt_sb), (psk, kt_sb)):
                    for h in range(HPC):
                        ps = ps_list[h]
                        tq = tmpp.tile([128, TS], F32, tag="tmp")
                        nc.scalar.copy(tq[:], ps[:])
                        tc_cos = tmpp.tile([128, TS], F32, tag="tmp")
                        nc.vector.tensor_mul(tc_cos[:], ps[:], cs[:])
                        tqs = tmpp.tile([128, TS], F32, tag="tmp")
                        nc.sync.dma_start(out=tqs[0:64, :], in_=tq[64:128, :])
                        nc.sync.dma_start(out=tqs[64:128, :], in_=tq[0:64, :])
                        nc.vector.tensor_mul(tqs[:], tqs[:], sn[:])
                        nc.vector.tensor_add(
                            dst[:, h, s0 : s0 + TS], tc_cos[:], tqs[:]
                        )

            if dbg is not None and b == 0:
                nc.sync.dma_start(out=dbg["dqt"][:], in_=qt_sb[:].bitcast(F32))
                nc.sync.dma_start(out=dbg["dkt"][:], in_=kt_sb[:].bitcast(F32))
                nc.sync.dma_start(out=dbg["dv"][:], in_=v_sb[:].bitcast(F32))

            # ---- phase B: causal attention per head ----
            for h in range(HPC):
                for qt in range(S // TQ):
                    q0 = qt * TQ
                    nvis = (q0 + TQ) // 128
                    pso = psump.tile([128, TQ], F32, tag="big")
                    psl = psump.tile([1, TQ], F32, tag="big")
                    for kb in range(nvis):
                        # trim the moving dim to the causal region (min 256
                        # wide so fp32r stays at full rate)
                        off = max(0, kb * 128 - q0)
                        off = min(off, TQ - 256)
                        W = TQ - off
                        pss = psump.tile([128, TQ], F32, tag="big")
                        nc.tensor.matmul(
                            pss[:, 0:W],
                            lhsT=(kt_sb[:, h, kb * 128 : (kb + 1) * 128]),
                            rhs=(qt_sb[:, h, q0 + off : q0 + TQ]),
                            start=True,
                            stop=True,
                        )
                        pt = ptp.tile([128, TQ], F32R, tag="pt")
                        nc.scalar.activation(
                            pt[:, 0:W], pss[:, 0:W], func=mybir.ActivationFunctionType.Exp
                        )
                        if kb * 128 + 127 > q0:
                            # diagonal block: zero future positions
                            nc.gpsimd.affine_select(
                                out=pt[:, 0:W],
                                in_=pt[:, 0:W],
                                pattern=[[1, W]],
                                base=q0 + off - kb * 128,
                                channel_multiplier=-1,
                                compare_op=mybir.AluOpType.is_ge,
                                fill=0.0,
                            )
                        first = kb == 0
                        last = kb == nvis - 1
                        nc.tensor.matmul(
                            pso[:, off:TQ],
                            lhsT=(v_sb[:, kb * DH + h * D : kb * DH + (h + 1) * D]),
                            rhs=(pt[:, 0:W]),
                            start=first,
                            stop=last,
                            skip_group_check=True,
                        )
                        nc.tensor.matmul(
                            psl[:, off:TQ],
                            lhsT=(ones_col[:]),
                            rhs=(pt[:, 0:W]),
                            start=first,
                            stop=last,
                            skip_group_check=True,
                        )
                        if dbg is not None and b == 0 and h == 0 and qt == 3:
                            nc.sync.dma_start(
                                out=dbg["dpt"][:, kb, 0:W], in_=pt[:, 0:W].bitcast(F32)
                            )
                            if off:
                                nc.gpsimd.memset(dbg["dpt"][:, kb, W:TQ], 0.0)
                    if dbg is not None and b == 0 and h == 0:
                        lrow = tmpp.tile([1, TQ], F32, tag="lrow", bufs=1)
                        nc.vector.tensor_copy(lrow[:], psl[:])
                        nc.sync.dma_start(out=dbg["dl"][:, q0 : q0 + TQ], in_=lrow[:])
                    lr = recp.tile([1, TQ], F32, tag="lr")
                    nc.scalar.copy(lr[:], psl[:])
                    rec = recp.tile([1, TQ], F32, tag="rec")
                    nc.vector.reciprocal_approx_fast(out=rec[:], in_=lr[:])
                    rb = tmpp.tile([128, TQ], F32, tag="tmp")
                    nc.gpsimd.partition_broadcast(rb[:], rec[:])
                    nc.vector.tensor_mul(at_sb[:, h, q0 : q0 + TQ], pso[:], rb[:])

            if dbg is not None and b == 0:
                nc.sync.dma_start(out=dbg["dat"][:], in_=at_sb[:].bitcast(F32))

            # ---- phase C: output projection (partial over local heads) ----
            if b == 0:
                nc.scalar.dma_start(out=wo_sb[:], in_=woTr[:])
            for sb in range(S // 128):
                for et in range(HID // 512):
                    psu = psump.tile([128, 512], F32, tag="big")
                    for h in range(HPC):
                        nc.tensor.matmul(
                            psu[:],
                            lhsT=(at_sb[:, h, sb * 128 : (sb + 1) * 128]),
                            rhs=(wo_sb[:, h, et * 512 : (et + 1) * 512]),
                            start=h == 0,
                            stop=h == HPC - 1,
                        )
                    ub = tmpp.tile([128, 512], F32, tag="tmp")
                    if (sb + et) % 2 == 0:
                        nc.scalar.copy(ub[:], psu[:])
                    else:
                        nc.vector.tensor_copy(ub[:], psu[:])
                    nc.sync.dma_start(
                        out=out[
                            bs + sb * 128 : bs + (sb + 1) * 128,
                            et * 512 : (et + 1) * 512,
                        ],
                        in_=ub[:],
                    )


def _host_inputs(hidden_states, cos, sin, wq, wk, wv, wo):
    x = np.ascontiguousarray(np.asarray(hidden_states, dtype=np.float32)).reshape(
        B * S, HID
    )
    xT = np.ascontiguousarray(x.T)
    cos = np.asarray(cos, dtype=np.float32)
    sin = np.asarray(sin, dtype=np.float32)
    # [D, B*S], column b*S+s = cos[b, s, :]
    cosT = np.ascontiguousarray(cos.reshape(B * S, D).T)
    sinT = np.ascontiguousarray(sin.reshape(B * S, D).T)
    sinT[: D // 2, :] *= -1.0  # fold rotate_half's negation into sin
    wq = np.asarray(wq, dtype=np.float32)
    wk = np.asarray(wk, dtype=np.float32)
    wv = np.asarray(wv, dtype=np.float32)
    wo = np.asarray(wo, dtype=np.float32)
    scale = 1.0 / math.sqrt(D)
    in_maps = []
    for c in range(NCORES):
        sl = slice(c * DH, (c + 1) * DH)
        in_maps.append(
            {
                "xT": xT,
                "wqT": np.ascontiguousarray(wq[sl].T * scale),
                "wkT": np.ascontiguousarray(wk[sl].T),
                "wvT": np.ascontiguousarray(wv[sl].T),
                "woT": np.ascontiguousarray(wo[:, sl].T),
                "cosT": cosT,
                "sinT": sinT,
            }
        )
    return in_maps


def kernel(
    hidden_states,
    cos,
    sin,
    wq,
    wk,
    wv,
    wo,
    position_ids=None,
    _trace=False,
    _tmpdir=None,
):
    global LAST_EXEC_TIME_NS
    if "nc" not in _CACHE:
        _CACHE["nc"] = _build_device_program()
    nc = _CACHE["nc"]
    in_maps = _host_inputs(hidden_states, cos, sin, wq, wk, wv, wo)
    res = run_bass_kernel_spmd(
        nc,
        in_maps,
        list(range(NCORES)),
        trace=_trace,
        tmpdir=_tmpdir,
    )
    LAST_EXEC_TIME_NS = res.exec_time_ns
    total = res.results[0]["out"].astype(np.float64)
    for c in range(1, NCORES):
        total += res.results[c]["out"]
    return total.astype(np.float32).reshape(B, S, HID)


# revision 14
# speedup vs baseline: 1.0005x; 1.0005x over previous
"""Tensor-parallel causal multi-head attention (RoPE) on 8 TRN2 NeuronCores.

Sharding: heads are split across the 8 cores (16 heads -> 2 heads/core).
wq/wk/wv are split column-wise (by output head), wo row-wise; hidden_states
is replicated.  Each core computes its 2 heads end-to-end (QKV projection,
RoPE, causal attention, output projection) and returns its additive partial
of the full output; the host sums the 8 partials.

Device-side layout choices (all matmuls contract over the partition dim):
  - X^T [HID, B*S] is produced on the host so projections need no on-device
    transposes.  Q and K are computed directly in transposed layout
    Q^T/K^T [d, s] (lhsT = W^T chunk, rhs = X^T chunk), V in normal layout
    [s, d] (lhsT = X^T chunk, rhs = W^T).
  - Scores are computed transposed: S^T[k, q] = (K^T chunk).T @ Q^T, so the
    exp'd probabilities P^T [k, q] feed the O^T = V.T @ P^T matmul directly
    with q as the 512-wide moving dim (full fp32r rate), no transposes.
  - softmax denominators l[q] = sum_k P^T[k, q] come from a ones-column
    matmul accumulated alongside O^T; 1/l is broadcast across partitions
    with a rank-1 fp32 matmul (ones_row.T @ recip_l).
  - No max-subtraction: scores are O(1) for this problem so exp is safe.
  - RoPE's rotate_half is a partition swap done with two SBUF->SBUF DMAs;
    the sign flip is folded into the host-prepared sin^T (lower half
    negated), and the 1/sqrt(D) score scale is folded into wq.
"""

import math

import numpy as np

import concourse.bass as bass
import concourse.tile as tile
from concourse import bacc, mybir
from concourse.bass_utils import run_bass_kernel_spmd

B, S, HID = 2, 2048, 2048
H, D = 16, 128
NCORES = 8
HPC = H // NCORES  # heads per core
DH = HPC * D  # per-core projection width (256)
NHC = HID // 128  # hid chunks (16)
TS = 512  # s-tile for projections
TQ = 512  # q-tile for attention
NKB = S // 128  # k blocks per sequence (16)
F32 = mybir.dt.float32
F32R = mybir.dt.float32r

LAST_EXEC_TIME_NS = None
_CACHE = {}


def _build_device_program():
    nc = bacc.Bacc(
        "TRN2",
        target_bir_lowering=False,
        debug=False,
        enable_asserts=False,
        num_devices=NCORES,
    )
    xT = nc.dram_tensor("xT", [HID, B * S], F32R, kind="ExternalInput").ap()
    wqT = nc.dram_tensor("wqT", [HID, DH], F32R, kind="ExternalInput").ap()
    wkT = nc.dram_tensor("wkT", [HID, DH], F32R, kind="ExternalInput").ap()
    wvT = nc.dram_tensor("wvT", [HID, DH], F32R, kind="ExternalInput").ap()
    woT = nc.dram_tensor("woT", [DH, HID], F32R, kind="ExternalInput").ap()
    cosT = nc.dram_tensor("cosT", [D, B * S], F32, kind="ExternalInput").ap()
    sinT = nc.dram_tensor("sinT", [D, B * S], F32, kind="ExternalInput").ap()
    out = nc.dram_tensor("out", [B * S, HID], F32, kind="ExternalOutput").ap()

    with tile.TileContext(nc) as tc:
        _emit_kernel(tc, xT, wqT, wkT, wvT, woT, cosT, sinT, out)

    nc.compile()
    return nc


def _emit_kernel(tc, xT, wqT, wkT, wvT, woT, cosT, sinT, out, dbg=None):
    from contextlib import ExitStack

    nc = tc.nc
    with ExitStack() as ctx:
        xTr = xT.rearrange("(hc p) s -> p hc s", p=128)  # [128, 16, B*S]
        wqTr = wqT.rearrange("(hc p) d -> p hc d", p=128)  # [128, 16, DH]
        wkTr = wkT.rearrange("(hc p) d -> p hc d", p=128)
        wvTr = wvT.rearrange("(hc p) d -> p hc d", p=128)
        woTr = woT.rearrange("(wc p) e -> p wc e", p=128)  # [128, HPC, HID]

        const = ctx.enter_context(tc.tile_pool(name="const", bufs=1))
        batchp = ctx.enter_context(tc.tile_pool(name="batchp", bufs=1))
        xtp = ctx.enter_context(tc.tile_pool(name="xtp", bufs=2))
        csp = ctx.enter_context(tc.tile_pool(name="csp", bufs=2))
        tmpp = ctx.enter_context(tc.tile_pool(name="tmpp", bufs=8))
        ptp = ctx.enter_context(tc.tile_pool(name="ptp", bufs=4))
        recp = ctx.enter_context(tc.tile_pool(name="recp", bufs=2))
        psump = ctx.enter_context(tc.tile_pool(name="psump", bufs=8, space="PSUM"))

        # ---- resident constants ----
        # split weight loads per hid-chunk group so the first matmuls only
        # wait for the chunks they read; wo is loaded later (phase C)
        wq_sb = const.tile([128, NHC, DH], F32R)
        wk_sb = const.tile([128, NHC, DH], F32R)
        wv_sb = const.tile([128, NHC, DH], F32R)
        for j in range(8):
            c0, c1 = j * 2, j * 2 + 2
            nc.scalar.dma_start(out=wq_sb[:, c0:c1, :], in_=wqTr[:, c0:c1, :])
            nc.scalar.dma_start(out=wk_sb[:, c0:c1, :], in_=wkTr[:, c0:c1, :])
            nc.scalar.dma_start(out=wv_sb[:, c0:c1, :], in_=wvTr[:, c0:c1, :])
        wo_sb = const.tile([128, HPC, HID], F32R)
        ones_f = const.tile([128, 1], F32)
        nc.vector.memset(ones_f[:], 1.0)
        ones_col = const.tile([128, 1], F32R)
        nc.scalar.copy(ones_col[:], ones_f[:])

        for b in range(B):
            bs = b * S
            # per-batch on-chip tensors (slots shared across batches via tags)
            qt_sb = batchp.tile([128, HPC, S], F32R, tag="qt")  # Q^T (scaled, roped)
            kt_sb = batchp.tile([128, HPC, S], F32R, tag="kt")  # K^T (roped)
            v_sb = batchp.tile([128, NKB * DH], F32R, tag="v")  # V row-blocks
            at_sb = batchp.tile([128, HPC, S], F32R, tag="at")  # attn out (A^T)

            # ---- phase A: QKV projections + RoPE ----
            for st in range(S // TS):
                s0 = st * TS
                psq = [
                    psump.tile([128, TS], F32, tag="big", name=f"psq{h}")
                    for h in range(HPC)
                ]
                psk = [
                    psump.tile([128, TS], F32, tag="big", name=f"psk{h}")
                    for h in range(HPC)
                ]
                psv = [
                    psump.tile([128, TS], F32, tag="big", name=f"psv{sp}")
                    for sp in range(TS // 256)
                ]
                for half in range(2):
                    xt = xtp.tile([128, 8, TS], F32R)
                    for xj in range(2):
                        nc.sync.dma_start(
                            out=xt[:, xj * 4 : xj * 4 + 4, :],
                            in_=xTr[
                                :,
                                half * 8 + xj * 4 : half * 8 + xj * 4 + 4,
                                bs + s0 : bs + s0 + TS,
                            ],
                        )
                    for i in range(8):
                        hc = half * 8 + i
                        first = hc == 0
                        last = hc == NHC - 1
                        for h in range(HPC):
                            nc.tensor.matmul(
                                psq[h][:],
                                lhsT=(wq_sb[:, hc, h * D : (h + 1) * D]),
                                rhs=(xt[:, i, :]),
                                start=first,
                                stop=last,
                            )
                            nc.tensor.matmul(
                                psk[h][:],
                                lhsT=(wk_sb[:, hc, h * D : (h + 1) * D]),
                                rhs=(xt[:, i, :]),
                                start=first,
                                stop=last,
                            )
                        for sp in range(TS // 256):
                            for sblk in range(2):
                                # one accumulation group per PSUM bank:
                                # start=True clears the whole bank, so only
                                # the first matmul touching the tile starts
                                nc.tensor.matmul(
                                    psv[sp][:, sblk * DH : (sblk + 1) * DH],
                                    lhsT=(
                                        xt[:, i, (sp * 2 + sblk) * 128 : (sp * 2 + sblk + 1) * 128]
                                    ),
                                    rhs=(wv_sb[:, hc, :]),
                                    start=first and sblk == 0,
                                    stop=last and sblk == 1,
                                    skip_group_check=True,
                                )
                # V: evacuate PSUM -> v_sb
                for sp in range(TS // 256):
                    blk0 = s0 // 128 + sp * 2
                    nc.scalar.copy(
                        v_sb[:, blk0 * DH : (blk0 + 2) * DH], psv[sp][:]
                    )
                # RoPE for Q and K
                cs = csp.tile([128, TS], F32, tag="cs")
                nc.sync.dma_start(out=cs[:], in_=cosT[:, bs + s0 : bs + s0 + TS])
                sn = csp.tile([128, TS], F32, tag="cs")
                nc.sync.dma_start(out=sn[:], in_=sinT[:, bs + s0 : bs + s0 + TS])
                for ps_list, dst in ((psq, qt_sb), (psk, kt_sb)):
                    for h in range(HPC):
                        ps = ps_list[h]
                        tq = tmpp.tile([128, TS], F32, tag="tmp")
                        nc.scalar.copy(tq[:], ps[:])
                        tc_cos = tmpp.tile([128, TS], F32, tag="tmp")
                        nc.vector.tensor_mul(tc_cos[:], ps[:], cs[:])
                        tqs = tmpp.tile([128, TS], F32, tag="tmp")
                        nc.sync.dma_start(out=tqs[0:64, :], in_=tq[64:128, :])
                        nc.sync.dma_start(out=tqs[64:128, :], in_=tq[0:64, :])
                        nc.vector.tensor_mul(tqs[:], tqs[:], sn[:])
                        nc.vector.tensor_add(
                            dst[:, h, s0 : s0 + TS], tc_cos[:], tqs[:]
                        )

            if dbg is not None and b == 0:
                nc.sync.dma_start(out=dbg["dqt"][:], in_=qt_sb[:].bitcast(F32))
                nc.sync.dma_start(out=dbg["dkt"][:], in_=kt_sb[:].bitcast(F32))
                nc.sync.dma_start(out=dbg["dv"][:], in_=v_sb[:].bitcast(F32))

            # ---- phase B: causal attention per head ----
            for h in range(HPC):
                for qt in range(S // TQ):
                    q0 = qt * TQ
                    nvis = (q0 + TQ) // 128
                    pso = psump.tile([128, TQ], F32, tag="big")
                    psl = psump.tile([1, TQ], F32, tag="big")
                    for kb in range(nvis):
                        # trim the moving dim to the causal region (min 256
                        # wide so fp32r stays at full rate)
                        off = max(0, kb * 128 - q0)
                        off = min(off, TQ - 256)
                        W = TQ - off
                        pss = psump.tile([128, TQ], F32, tag="big")
                        nc.tensor.matmul(
                            pss[:, 0:W],
                            lhsT=(kt_sb[:, h, kb * 128 : (kb + 1) * 128]),
                            rhs=(qt_sb[:, h, q0 + off : q0 + TQ]),
                            start=True,
                            stop=True,
                        )
                        pt = ptp.tile([128, TQ], F32R, tag="pt")
                        nc.scalar.activation(
                            pt[:, 0:W], pss[:, 0:W], func=mybir.ActivationFunctionType.Exp
                        )
                        if kb * 128 + 127 > q0:
                            # diagonal block: zero future positions
                            nc.gpsimd.affine_select(
                                out=pt[:, 0:W],
                                in_=pt[:, 0:W],
                                pattern=[[1, W]],
                                base=q0 + off - kb * 128,
                                channel_multiplier=-1,
                                compare_op=mybir.AluOpType.is_ge,
                                fill=0.0,
                            )
                        first = kb == 0
                        last = kb == nvis - 1
                        nc.tensor.matmul(
                            pso[:, off:TQ],
                            lhsT=(v_sb[:, kb * DH + h * D : kb * DH + (h + 1) * D]),
                            rhs=(pt[:, 0:W]),
                            start=first,
                            stop=last,
                            skip_group_check=True,
                        )
                        nc.tensor.matmul(
                            psl[:, off:TQ],
                            lhsT=(ones_col[:]),
                            rhs=(pt[:, 0:W]),
                            start=first,
                            stop=last,
                            skip_group_check=True,
                        )
                        if dbg is not None and b == 0 and h == 0 and qt == 3:
                            nc.sync.dma_start(
                                out=dbg["dpt"][:, kb, 0:W], in_=pt[:, 0:W].bitcast(F32)
                            )
                            if off:
                                nc.gpsimd.memset(dbg["dpt"][:, kb, W:TQ], 0.0)
                    if dbg is not None and b == 0 and h == 0:
                        lrow = tmpp.tile([1, TQ], F32, tag="lrow", bufs=1)
                        nc.vector.tensor_copy(lrow[:], psl[:])
                        nc.sync.dma_start(out=dbg["dl"][:, q0 : q0 + TQ], in_=lrow[:])
                    lr = recp.tile([1, TQ], F32, tag="lr")
                    nc.scalar.copy(lr[:], psl[:])
                    rec = recp.tile([1, TQ], F32, tag="rec")
                    nc.vector.reciprocal_approx_fast(out=rec[:], in_=lr[:])
                    rb = tmpp.tile([128, TQ], F32, tag="tmp")
                    nc.gpsimd.partition_broadcast(rb[:], rec[:])
                    nc.vector.tensor_mul(at_sb[:, h, q0 : q0 + TQ], pso[:], rb[:])

            if dbg is not None and b == 0:
                nc.sync.dma_start(out=dbg["dat"][:], in_=at_sb[:].bitcast(F32))

            # ---- phase C: output projection (partial over local heads) ----
            if b == 0:
                nc.scalar.dma_start(out=wo_sb[:], in_=woTr[:])
            for sb in range(S // 128):
                # interleave the 4 e-tile accumulations so consecutive
                # matmuls never target the same PSUM region (which would
                # serialize fill against drain)
                psus = [
                    psump.tile([128, 512], F32, tag="big", name=f"psu{et}")
                    for et in range(HID // 512)
                ]
                for h in range(HPC):
                    for et in range(HID // 512):
                        nc.tensor.matmul(
                            psus[et][:],
                            lhsT=(at_sb[:, h, sb * 128 : (sb + 1) * 128]),
                            rhs=(wo_sb[:, h, et * 512 : (et + 1) * 512]),
                            start=h == 0,
                            stop=h == HPC - 1,
                        )
                for et in range(HID // 512):
                    ub = tmpp.tile([128, 512], F32, tag="tmp")
                    if (sb + et) % 2 == 0:
                        nc.scalar.copy(ub[:], psus[et][:])
                    else:
                        nc.vector.tensor_copy(ub[:], psus[et][:])
                    nc.sync.dma_start(
                        out=out[
                            bs + sb * 128 : bs + (sb + 1) * 128,
                            et * 512 : (et + 1) * 512,
                        ],
                        in_=ub[:],
                    )


def _host_inputs(hidden_states, cos, sin, wq, wk, wv, wo):
    x = np.ascontiguousarray(np.asarray(hidden_states, dtype=np.float32)).reshape(
        B * S, HID
    )
    xT = np.ascontiguousarray(x.T)
    cos = np.asarray(cos, dtype=np.float32)
    sin = np.asarray(sin, dtype=np.float32)
    # [D, B*S], column b*S+s = cos[b, s, :]
    cosT = np.ascontiguousarray(cos.reshape(B * S, D).T)
    sinT = np.ascontiguousarray(sin.reshape(B * S, D).T)
    sinT[: D // 2, :] *= -1.0  # fold rotate_half's negation into sin
    wq = np.asarray(wq, dtype=np.float32)
    wk = np.asarray(wk, dtype=np.float32)
    wv = np.asarray(wv, dtype=np.float32)
    wo = np.asarray(wo, dtype=np.float32)
    scale = 1.0 / math.sqrt(D)
    in_maps = []
    for c in range(NCORES):
        sl = slice(c * DH, (c + 1) * DH)
        in_maps.append(
            {
                "xT": xT,
                "wqT": np.ascontiguousarray(wq[sl].T * scale),
                "wkT": np.ascontiguousarray(wk[sl].T),
                "wvT": np.ascontiguousarray(wv[sl].T),
                "woT": np.ascontiguousarray(wo[:, sl].T),
                "cosT": cosT,
                "sinT": sinT,
            }
        )
    return in_maps


def kernel(
    hidden_states,
    cos,
    sin,
    wq,
    wk,
    wv,
    wo,
    position_ids=None,
    _trace=False,
    _tmpdir=None,
):
    global LAST_EXEC_TIME_NS
    if "nc" not in _CACHE:
        _CACHE["nc"] = _build_device_program()
    nc = _CACHE["nc"]
    in_maps = _host_inputs(hidden_states, cos, sin, wq, wk, wv, wo)
    res = run_bass_kernel_spmd(
        nc,
        in_maps,
        list(range(NCORES)),
        trace=_trace,
        tmpdir=_tmpdir,
    )
    LAST_EXEC_TIME_NS = res.exec_time_ns
    total = res.results[0]["out"].astype(np.float64)
    for c in range(1, NCORES):
        total += res.results[c]["out"]
    return total.astype(np.float32).reshape(B, S, HID)


# revision 15
# speedup vs baseline: 1.0068x; 1.0063x over previous
"""Tensor-parallel causal multi-head attention (RoPE) on 8 TRN2 NeuronCores.

Sharding: heads are split across the 8 cores (16 heads -> 2 heads/core).
wq/wk/wv are split column-wise (by output head), wo row-wise; hidden_states
is replicated.  Each core computes its 2 heads end-to-end (QKV projection,
RoPE, causal attention, output projection) and returns its additive partial
of the full output; the host sums the 8 partials.

Device-side layout choices (all matmuls contract over the partition dim):
  - X^T [HID, B*S] is produced on the host so projections need no on-device
    transposes.  Q and K are computed directly in transposed layout
    Q^T/K^T [d, s] (lhsT = W^T chunk, rhs = X^T chunk), V in normal layout
    [s, d] (lhsT = X^T chunk, rhs = W^T).
  - Scores are computed transposed: S^T[k, q] = (K^T chunk).T @ Q^T, so the
    exp'd probabilities P^T [k, q] feed the O^T = V.T @ P^T matmul directly
    with q as the 512-wide moving dim (full fp32r rate), no transposes.
  - softmax denominators l[q] = sum_k P^T[k, q] come from a ones-column
    matmul accumulated alongside O^T; 1/l is broadcast across partitions
    with a rank-1 fp32 matmul (ones_row.T @ recip_l).
  - No max-subtraction: scores are O(1) for this problem so exp is safe.
  - RoPE's rotate_half is a partition swap done with two SBUF->SBUF DMAs;
    the sign flip is folded into the host-prepared sin^T (lower half
    negated), and the 1/sqrt(D) score scale is folded into wq.
"""

import math

import numpy as np

import concourse.bass as bass
import concourse.tile as tile
from concourse import bacc, mybir
from concourse.bass_utils import run_bass_kernel_spmd

B, S, HID = 2, 2048, 2048
H, D = 16, 128
NCORES = 8
HPC = H // NCORES  # heads per core
DH = HPC * D  # per-core projection width (256)
NHC = HID // 128  # hid chunks (16)
TS = 512  # s-tile for projections
TQ = 512  # q-tile for attention
NKB = S // 128  # k blocks per sequence (16)
F32 = mybir.dt.float32
F32R = mybir.dt.float32r

LAST_EXEC_TIME_NS = None
_CACHE = {}


def _build_device_program():
    nc = bacc.Bacc(
        "TRN2",
        target_bir_lowering=False,
        debug=False,
        enable_asserts=False,
        num_devices=NCORES,
    )
    xT = nc.dram_tensor("xT", [HID, B * S], F32R, kind="ExternalInput").ap()
    wqT = nc.dram_tensor("wqT", [HID, DH], F32R, kind="ExternalInput").ap()
    wkT = nc.dram_tensor("wkT", [HID, DH], F32R, kind="ExternalInput").ap()
    wvT = nc.dram_tensor("wvT", [HID, DH], F32R, kind="ExternalInput").ap()
    woT = nc.dram_tensor("woT", [DH, HID], F32R, kind="ExternalInput").ap()
    cosT = nc.dram_tensor("cosT", [D, B * S], F32, kind="ExternalInput").ap()
    sinT = nc.dram_tensor("sinT", [D, B * S], F32, kind="ExternalInput").ap()
    out = nc.dram_tensor("out", [B * S, HID], F32, kind="ExternalOutput").ap()

    with tile.TileContext(nc) as tc:
        _emit_kernel(tc, xT, wqT, wkT, wvT, woT, cosT, sinT, out)

    nc.compile()
    return nc


def _emit_kernel(tc, xT, wqT, wkT, wvT, woT, cosT, sinT, out, dbg=None):
    from contextlib import ExitStack

    nc = tc.nc
    with ExitStack() as ctx:
        xTr = xT.rearrange("(hc p) s -> p hc s", p=128)  # [128, 16, B*S]
        wqTr = wqT.rearrange("(hc p) d -> p hc d", p=128)  # [128, 16, DH]
        wkTr = wkT.rearrange("(hc p) d -> p hc d", p=128)
        wvTr = wvT.rearrange("(hc p) d -> p hc d", p=128)
        woTr = woT.rearrange("(wc p) e -> p wc e", p=128)  # [128, HPC, HID]

        const = ctx.enter_context(tc.tile_pool(name="const", bufs=1))
        batchp = ctx.enter_context(tc.tile_pool(name="batchp", bufs=1))
        xtp = ctx.enter_context(tc.tile_pool(name="xtp", bufs=2))
        csp = ctx.enter_context(tc.tile_pool(name="csp", bufs=2))
        tmpp = ctx.enter_context(tc.tile_pool(name="tmpp", bufs=8))
        ptp = ctx.enter_context(tc.tile_pool(name="ptp", bufs=4))
        recp = ctx.enter_context(tc.tile_pool(name="recp", bufs=2))
        psump = ctx.enter_context(tc.tile_pool(name="psump", bufs=8, space="PSUM"))

        # ---- resident constants ----
        # split weight loads per hid-chunk group so the first matmuls only
        # wait for the chunks they read; wo is loaded later (phase C)
        wq_sb = const.tile([128, NHC, DH], F32R)
        wk_sb = const.tile([128, NHC, DH], F32R)
        wv_sb = const.tile([128, NHC, DH], F32R)
        for j in range(8):
            c0, c1 = j * 2, j * 2 + 2
            nc.scalar.dma_start(out=wq_sb[:, c0:c1, :], in_=wqTr[:, c0:c1, :])
            nc.scalar.dma_start(out=wk_sb[:, c0:c1, :], in_=wkTr[:, c0:c1, :])
            nc.scalar.dma_start(out=wv_sb[:, c0:c1, :], in_=wvTr[:, c0:c1, :])
        wo_sb = const.tile([128, HPC, HID], F32R)
        ones_f = const.tile([128, 1], F32)
        nc.vector.memset(ones_f[:], 1.0)
        ones_col = const.tile([128, 1], F32R)
        nc.scalar.copy(ones_col[:], ones_f[:])

        for b in range(B):
            bs = b * S
            # per-batch on-chip tensors (slots shared across batches via tags)
            qt_sb = batchp.tile([128, HPC, S], F32R, tag="qt")  # Q^T (scaled, roped)
            kt_sb = batchp.tile([128, HPC, S], F32R, tag="kt")  # K^T (roped)
            v_sb = batchp.tile([128, NKB * DH], F32R, tag="v")  # V row-blocks
            at_sb = batchp.tile([128, HPC, S], F32R, tag="at")  # attn out (A^T)

            # ---- phase A: QKV projections + RoPE ----
            for st in range(S // TS):
                s0 = st * TS
                psq = [
                    psump.tile([128, TS], F32, tag="big", name=f"psq{h}")
                    for h in range(HPC)
                ]
                psk = [
                    psump.tile([128, TS], F32, tag="big", name=f"psk{h}")
                    for h in range(HPC)
                ]
                psv = [
                    psump.tile([128, TS], F32, tag="big", name=f"psv{sp}")
                    for sp in range(TS // 256)
                ]
                for half in range(2):
                    xt = xtp.tile([128, 8, TS], F32R)
                    for xj in range(2):
                        nc.sync.dma_start(
                            out=xt[:, xj * 4 : xj * 4 + 4, :],
                            in_=xTr[
                                :,
                                half * 8 + xj * 4 : half * 8 + xj * 4 + 4,
                                bs + s0 : bs + s0 + TS,
                            ],
                        )
                    for i in range(8):
                        hc = half * 8 + i
                        first = hc == 0
                        last = hc == NHC - 1
                        for h in range(HPC):
                            nc.tensor.matmul(
                                psq[h][:],
                                lhsT=(wq_sb[:, hc, h * D : (h + 1) * D]),
                                rhs=(xt[:, i, :]),
                                start=first,
                                stop=last,
                            )
                            nc.tensor.matmul(
                                psk[h][:],
                                lhsT=(wk_sb[:, hc, h * D : (h + 1) * D]),
                                rhs=(xt[:, i, :]),
                                start=first,
                                stop=last,
                            )
                        for sp in range(TS // 256):
                            for sblk in range(2):
                                # one accumulation group per PSUM bank:
                                # start=True clears the whole bank, so only
                                # the first matmul touching the tile starts
                                nc.tensor.matmul(
                                    psv[sp][:, sblk * DH : (sblk + 1) * DH],
                                    lhsT=(
                                        xt[:, i, (sp * 2 + sblk) * 128 : (sp * 2 + sblk + 1) * 128]
                                    ),
                                    rhs=(wv_sb[:, hc, :]),
                                    start=first and sblk == 0,
                                    stop=last and sblk == 1,
                                    skip_group_check=True,
                                )
                # V: evacuate PSUM -> v_sb
                for sp in range(TS // 256):
                    blk0 = s0 // 128 + sp * 2
                    nc.scalar.copy(
                        v_sb[:, blk0 * DH : (blk0 + 2) * DH], psv[sp][:]
                    )
                # RoPE for Q and K
                cs = csp.tile([128, TS], F32, tag="cs")
                nc.sync.dma_start(out=cs[:], in_=cosT[:, bs + s0 : bs + s0 + TS])
                sn = csp.tile([128, TS], F32, tag="cs")
                nc.sync.dma_start(out=sn[:], in_=sinT[:, bs + s0 : bs + s0 + TS])
                for ps_list, dst in ((psq, qt_sb), (psk, kt_sb)):
                    for h in range(HPC):
                        ps = ps_list[h]
                        tq = tmpp.tile([128, TS], F32, tag="tmp")
                        nc.scalar.copy(tq[:], ps[:])
                        tc_cos = tmpp.tile([128, TS], F32, tag="tmp")
                        nc.vector.tensor_mul(tc_cos[:], ps[:], cs[:])
                        tqs = tmpp.tile([128, TS], F32, tag="tmp")
                        nc.sync.dma_start(out=tqs[0:64, :], in_=tq[64:128, :])
                        nc.sync.dma_start(out=tqs[64:128, :], in_=tq[0:64, :])
                        nc.vector.tensor_mul(tqs[:], tqs[:], sn[:])
                        nc.vector.tensor_add(
                            dst[:, h, s0 : s0 + TS], tc_cos[:], tqs[:]
                        )

            if dbg is not None and b == 0:
                nc.sync.dma_start(out=dbg["dqt"][:], in_=qt_sb[:].bitcast(F32))
                nc.sync.dma_start(out=dbg["dkt"][:], in_=kt_sb[:].bitcast(F32))
                nc.sync.dma_start(out=dbg["dv"][:], in_=v_sb[:].bitcast(F32))

            # ---- phase B: causal attention per head ----
            for h in range(HPC):
                for qt in range(S // TQ):
                    q0 = qt * TQ
                    nvis = (q0 + TQ) // 128
                    pso = psump.tile([128, TQ], F32, tag="big")
                    psl = psump.tile([1, TQ], F32, tag="big")
                    for kb in range(nvis):
                        # trim the moving dim to the causal region (min 256
                        # wide so fp32r stays at full rate)
                        off = max(0, kb * 128 - q0)
                        off = min(off, TQ - 256)
                        W = TQ - off
                        pss = psump.tile([128, TQ], F32, tag="big")
                        nc.tensor.matmul(
                            pss[:, 0:W],
                            lhsT=(kt_sb[:, h, kb * 128 : (kb + 1) * 128]),
                            rhs=(qt_sb[:, h, q0 + off : q0 + TQ]),
                            start=True,
                            stop=True,
                        )
                        pt = ptp.tile([128, TQ], F32R, tag="pt")
                        nc.scalar.activation(
                            pt[:, 0:W], pss[:, 0:W], func=mybir.ActivationFunctionType.Exp
                        )
                        if kb * 128 + 127 > q0:
                            # diagonal block: zero future positions
                            nc.gpsimd.affine_select(
                                out=pt[:, 0:W],
                                in_=pt[:, 0:W],
                                pattern=[[1, W]],
                                base=q0 + off - kb * 128,
                                channel_multiplier=-1,
                                compare_op=mybir.AluOpType.is_ge,
                                fill=0.0,
                            )
                        first = kb == 0
                        last = kb == nvis - 1
                        nc.tensor.matmul(
                            pso[:, off:TQ],
                            lhsT=(v_sb[:, kb * DH + h * D : kb * DH + (h + 1) * D]),
                            rhs=(pt[:, 0:W]),
                            start=first,
                            stop=last,
                            skip_group_check=True,
                        )
                        nc.tensor.matmul(
                            psl[:, off:TQ],
                            lhsT=(ones_col[:]),
                            rhs=(pt[:, 0:W]),
                            start=first,
                            stop=last,
                            skip_group_check=True,
                        )
                        if dbg is not None and b == 0 and h == 0 and qt == 3:
                            nc.sync.dma_start(
                                out=dbg["dpt"][:, kb, 0:W], in_=pt[:, 0:W].bitcast(F32)
                            )
                            if off:
                                nc.gpsimd.memset(dbg["dpt"][:, kb, W:TQ], 0.0)
                    if dbg is not None and b == 0 and h == 0:
                        lrow = tmpp.tile([1, TQ], F32, tag="lrow", bufs=1)
                        nc.vector.tensor_copy(lrow[:], psl[:])
                        nc.sync.dma_start(out=dbg["dl"][:, q0 : q0 + TQ], in_=lrow[:])
                    lr = recp.tile([1, TQ], F32, tag="lr")
                    nc.scalar.copy(lr[:], psl[:])
                    rec = recp.tile([1, TQ], F32, tag="rec")
                    nc.vector.reciprocal_approx_fast(out=rec[:], in_=lr[:])
                    rb = tmpp.tile([128, TQ], F32, tag="tmp")
                    nc.gpsimd.partition_broadcast(rb[:], rec[:])
                    nc.vector.tensor_mul(at_sb[:, h, q0 : q0 + TQ], pso[:], rb[:])

            if dbg is not None and b == 0:
                nc.sync.dma_start(out=dbg["dat"][:], in_=at_sb[:].bitcast(F32))

            # ---- phase C: output projection (partial over local heads) ----
            if b == 0:
                nc.scalar.dma_start(out=wo_sb[:], in_=woTr[:])
            for sb in range(S // 128):
                # interleave the 4 e-tile accumulations so consecutive
                # matmuls never target the same PSUM region (which would
                # serialize fill against drain)
                psus = [
                    psump.tile([128, 512], F32, tag="big", name=f"psu{et}")
                    for et in range(HID // 512)
                ]
                for h in range(HPC):
                    for et in range(HID // 512):
                        nc.tensor.matmul(
                            psus[et][:],
                            lhsT=(at_sb[:, h, sb * 128 : (sb + 1) * 128]),
                            rhs=(wo_sb[:, h, et * 512 : (et + 1) * 512]),
                            start=h == 0,
                            stop=h == HPC - 1,
                        )
                for et in range(HID // 512):
                    ub = tmpp.tile([128, 512], F32, tag="tmp")
                    nc.scalar.copy(ub[:, 0:256], psus[et][:, 0:256])
                    nc.vector.tensor_copy(ub[:, 256:512], psus[et][:, 256:512])
                    nc.sync.dma_start(
                        out=out[
                            bs + sb * 128 : bs + (sb + 1) * 128,
                            et * 512 : (et + 1) * 512,
                        ],
                        in_=ub[:],
                    )


def _host_inputs(hidden_states, cos, sin, wq, wk, wv, wo):
    x = np.ascontiguousarray(np.asarray(hidden_states, dtype=np.float32)).reshape(
        B * S, HID
    )
    xT = np.ascontiguousarray(x.T)
    cos = np.asarray(cos, dtype=np.float32)
    sin = np.asarray(sin, dtype=np.float32)
    # [D, B*S], column b*S+s = cos[b, s, :]
    cosT = np.ascontiguousarray(cos.reshape(B * S, D).T)
    sinT = np.ascontiguousarray(sin.reshape(B * S, D).T)
    sinT[: D // 2, :] *= -1.0  # fold rotate_half's negation into sin
    wq = np.asarray(wq, dtype=np.float32)
    wk = np.asarray(wk, dtype=np.float32)
    wv = np.asarray(wv, dtype=np.float32)
    wo = np.asarray(wo, dtype=np.float32)
    scale = 1.0 / math.sqrt(D)
    in_maps = []
    for c in range(NCORES):
        sl = slice(c * DH, (c + 1) * DH)
        in_maps.append(
            {
                "xT": xT,
                "wqT": np.ascontiguousarray(wq[sl].T * scale),
                "wkT": np.ascontiguousarray(wk[sl].T),
                "wvT": np.ascontiguousarray(wv[sl].T),
                "woT": np.ascontiguousarray(wo[:, sl].T),
                "cosT": cosT,
                "sinT": sinT,
            }
        )
    return in_maps


def kernel(
    hidden_states,
    cos,
    sin,
    wq,
    wk,
    wv,
    wo,
    position_ids=None,
    _trace=False,
    _tmpdir=None,
):
    global LAST_EXEC_TIME_NS
    if "nc" not in _CACHE:
        _CACHE["nc"] = _build_device_program()
    nc = _CACHE["nc"]
    in_maps = _host_inputs(hidden_states, cos, sin, wq, wk, wv, wo)
    res = run_bass_kernel_spmd(
        nc,
        in_maps,
        list(range(NCORES)),
        trace=_trace,
        tmpdir=_tmpdir,
    )
    LAST_EXEC_TIME_NS = res.exec_time_ns
    total = res.results[0]["out"].astype(np.float64)
    for c in range(1, NCORES):
        total += res.results[c]["out"]
    return total.astype(np.float32).reshape(B, S, HID)
